# revision 14
# baseline (speedup 1.0000x reference)
"""Trainium2 Bass kernel: multi-head causal attention with RoPE.

Problem: x[2, 2048, 1024], w_qkv[3072, 1024], w_out[1024, 1024], b_out[1024];
16 heads, head_dim 64, causal softmax attention with rotate-half RoPE.

Sharding: 8 cores = 2 batch groups x 4 head-groups.  Core c handles batch
b = c // 4 and heads 4*(c%4) .. 4*(c%4)+3.  Each core computes q/k/v
projections for its 4 heads, RoPE, causal flash-style attention with
softmax, then the cores in a batch group AllGather the (normalized)
attention outputs per seq chunk ([256, 512] bf16 slices -> [1024, 512]),
and every core computes the FULL out-projection for its own 256 output
rows locally.  No ReduceScatter of 4x-larger partials, no output copies:
the AllGather moves 4x fewer bytes than the partial-sum RS would, and the
final tail only waits on one small AG of the last chunk's second head
pair.

On-chip layout notes:
 - Everything keeps seq-len on the free axis and features on partitions
   (q/k/v are computed directly in transposed [feat, s] layout), so attention
   scores come out as scoresT [j, i] and no transposes are ever needed.
 - Softmax skips the max-subtraction (scores are O(1) by construction),
   so exp() happens directly on the QK^T PSUM tile, and the denominators are
   accumulated by an all-ones row appended to v in the AV matmul.
 - bf16 operands everywhere on the PE; fp32 PSUM accumulation.
 - 2 heads per 128-partition span -> automatic PE row packing for the K=64
   QK^T matmuls (tile_position derived from base partitions).
 - ScalarE runs ONLY Exp (no activation-table switching); all evacuation
   copies run on VectorE.
 - The chunk loop is software-pipelined: projections for chunk c+1 and the
   out-projection of chunk c-1 are interleaved into attention(c) at
   t-iteration granularity so the PE never stalls on the softmax-normalize
   chain.
 - Startup: per-jf qkv weight slices go on the Scalar HWDGE queue while
   per-k x chunk-0 slices go on the Sync queue, so the first matmul starts
   as soon as the first two small DMAs land.
"""

import os
import sys

import numpy as np

for _p in ("/opt/trn_rl_repo", "/opt/pypackages"):
    if os.path.isdir(_p) and _p not in sys.path:
        sys.path.append(_p)

import ml_dtypes  # noqa: E402

import concourse.bass as bass  # noqa: E402,F401
import concourse.mybir as mybir  # noqa: E402
import concourse.tile as tile  # noqa: E402
from concourse import bacc  # noqa: E402
from concourse.bass_utils import run_bass_kernel_spmd  # noqa: E402

BF = ml_dtypes.bfloat16
F32 = mybir.dt.float32
BF16 = mybir.dt.bfloat16
AF = mybir.ActivationFunctionType
ALU = mybir.AluOpType

B, S, D, H, HD = 2, 2048, 1024, 16, 64
NCORES = 8
HPC = 4  # heads per core
GROUPS = [[0, 1, 2, 3], [4, 5, 6, 7]]
CH = 512  # seq chunk width
NCH = S // CH  # 4
KT = D // 128  # 8 contraction tiles for the projections
NJT = S // 128  # 16 key tiles
EOUT = D // 4  # 256 output-feature rows owned by each core
ROPE_BASE = 10000.0
SCALE = 1.0 / 8.0  # 1/sqrt(64)


def build_nc():
    nc = bacc.Bacc("TRN2", target_bir_lowering=False, debug=False, num_devices=NCORES)

    xT_d = nc.dram_tensor("xT", [D, S], BF16, kind="ExternalInput").ap()
    wqk4_d = nc.dram_tensor("wqk4", [4 * 128, KT, 128], BF16, kind="ExternalInput").ap()
    wv_d = nc.dram_tensor("wvT", [D, HPC * HD], BF16, kind="ExternalInput").ap()
    wo_d = nc.dram_tensor("woT", [D, EOUT], BF16, kind="ExternalInput").ap()
    cos_d = nc.dram_tensor("cos2", [128, S], BF16, kind="ExternalInput").ap()
    sin_d = nc.dram_tensor("sin2", [128, S], BF16, kind="ExternalInput").ap()
    tri_d = nc.dram_tensor("trim", [128, 128], BF16, kind="ExternalInput").ap()
    bias_d = nc.dram_tensor("bias2", [EOUT, 1], F32, kind="ExternalInput").ap()
    out_d = nc.dram_tensor("out", [EOUT, S], BF16, kind="ExternalOutput").ap()

    # AllGather buffers: chunks 0-2 gather all 4 heads at once; chunk 3 is
    # split by head pair so the first AG fires while pr=1 is still computing
    agin_d = [
        nc.dram_tensor(f"agin{c}", [2 * 128, CH], BF16).ap() for c in range(NCH - 1)
    ]
    agout_d = [
        nc.dram_tensor(f"agout{c}", [D, CH], BF16).ap() for c in range(NCH - 1)
    ]
    agin3_d = [nc.dram_tensor(f"agin3{h}", [128, CH], BF16).ap() for h in range(2)]
    agout3_d = [
        nc.dram_tensor(f"agout3{h}", [4 * 128, CH], BF16).ap() for h in range(2)
    ]

    xT_r = xT_d.rearrange("(k p) s -> p k s", p=128)
    wv_r = wv_d.rearrange("(k p) j -> p k j", p=128)
    wo_r = wo_d.rearrange("(k p) e -> p k e", p=128)
    out_r = out_d.rearrange("(t p) s -> p t s", p=128)
    agout_r = [a.rearrange("(k p) s -> p k s", p=128) for a in agout_d]
    agout3_r = [a.rearrange("(k p) s -> p k s", p=128) for a in agout3_d]

    with tile.TileContext(nc) as tc:
        with (
            tc.tile_pool(name="const", bufs=1) as cpool,
            tc.tile_pool(name="rope", bufs=3) as rpool,
            tc.tile_pool(name="attn", bufs=3) as apool,
            tc.tile_pool(name="evac", bufs=3) as epool,
            tc.tile_pool(name="sums", bufs=4) as spool,
            tc.tile_pool(name="agx", bufs=2) as agpool,
            tc.tile_pool(name="pmm", bufs=1, space="PSUM") as pmm,
            tc.tile_pool(name="ps", bufs=2, space="PSUM") as psp,
            tc.tile_pool(name="po", bufs=3, space="PSUM") as pop,
        ):
            # ---- startup loads.  Scalar HWDGE queue: qkv weights (per-jf
            # slices so the first matmul only waits on one 256KB transfer).
            # Sync HWDGE queue: x chunk 0 per-k slices, then the x rest as
            # one large transfer.  GpSimd SWDGE queue: rope tables, mask,
            # out-proj weights, bias. ----
            wqk_sb = cpool.tile([128, 4, KT, 128], BF16, tag="wqk")
            for jf in range(4):
                nc.scalar.dma_start(
                    wqk_sb[:, jf, :, :], wqk4_d[jf * 128 : (jf + 1) * 128, :, :]
                )
            wv_sb = cpool.tile([128, KT, 256], BF16, tag="wv")
            nc.scalar.dma_start(wv_sb[:, :, :], wv_r[:, :, :])

            xc0 = cpool.tile([128, KT, CH], BF16, tag="xc0")
            for k in range(KT):
                nc.sync.dma_start(xc0[:, k, :], xT_r[:, k, 0:CH])
            xrest = cpool.tile([128, KT, S - CH], BF16, tag="xrest")
            nc.sync.dma_start(xrest[:, :, :], xT_r[:, :, CH:S])

            def xcols(k, lo, hi):
                """x tile AP for contraction tile k, seq columns [lo, hi)."""
                if hi <= CH:
                    return xc0[:, k, lo:hi]
                return xrest[:, k, lo - CH : hi - CH]

            cos_sb = cpool.tile([128, S], BF16, tag="cos")
            nc.gpsimd.dma_start(cos_sb[:, :], cos_d)
            sin_sb = cpool.tile([128, S], BF16, tag="sin")
            nc.gpsimd.dma_start(sin_sb[:, :], sin_d)
            tri_sb = cpool.tile([128, 128], BF16, tag="tri")
            nc.gpsimd.dma_start(tri_sb[:, :], tri_d)
            wo_sb = cpool.tile([128, KT, EOUT], BF16, tag="wo")
            nc.gpsimd.dma_start(wo_sb[:, :, :], wo_r[:, :, :])
            bias_sb = cpool.tile([128, 2, 1], F32, tag="bias")
            nc.gpsimd.dma_start(bias_sb[:, :, :], bias_d.rearrange("(t p) o -> p t o", p=128))

            # ---- persistent activations (bf16) ----
            qT = [cpool.tile([128, S], BF16, tag=f"qT{i}", name=f"qT{i}") for i in range(2)]
            kT = [cpool.tile([128, S], BF16, tag=f"kT{i}", name=f"kT{i}") for i in range(2)]
            oT = [cpool.tile([128, S], BF16, tag=f"oT{i}", name=f"oT{i}") for i in range(2)]
            # v with an appended ones-column per head: [j_part, jt, head, 65]
            v_sb = cpool.tile([128, NJT, HPC, HD + 1], BF16, tag="v")
            nc.vector.memset(v_sb[:, :, :, HD : HD + 1], 1.0)
            agx3 = [
                cpool.tile([128, 4, CH], BF16, tag=f"agx3{h}", name=f"agx3{h}")
                for h in range(2)
            ]

            def qkproj_unit(c, jf):
                cs = slice(c * CH, (c + 1) * CH)
                dest = (qT[0], qT[1], kT[0], kT[1])[jf]
                ps = pmm.tile([128, CH], F32, tag="pmm", name="psqk")
                for k in range(KT):
                    nc.tensor.matmul(
                        ps[:, :],
                        wqk_sb[:, jf, k, :],
                        xcols(k, c * CH, (c + 1) * CH),
                        start=(k == 0),
                        stop=(k == KT - 1),
                    )
                qraw = rpool.tile([128, CH], BF16, tag="qraw")
                nc.vector.tensor_copy(qraw[:, :], ps[:, :])
                # rotate-half: qsw[r] = qraw[r ^ 32] (sign absorbed in sin2)
                qsw = rpool.tile([128, CH], BF16, tag="qsw")
                for h2 in (0, 64):
                    nc.vector.tensor_copy(
                        qsw[h2 : h2 + 32, :], qraw[h2 + 32 : h2 + 64, :]
                    )
                    nc.vector.tensor_copy(
                        qsw[h2 + 32 : h2 + 64, :], qraw[h2 : h2 + 32, :]
                    )
                t1 = rpool.tile([128, CH], BF16, tag="t1")
                nc.vector.tensor_mul(t1[:, :], qraw[:, :], cos_sb[:, cs])
                t2 = rpool.tile([128, CH], BF16, tag="t2")
                nc.vector.tensor_mul(t2[:, :], qsw[:, :], sin_sb[:, cs])
                nc.vector.tensor_add(dest[:, cs], t1[:, :], t2[:, :])

            def vproj_unit(c, sub):
                jt = 4 * c + sub
                s0 = c * CH + sub * 128
                pv = pmm.tile([128, CH], F32, tag="pmm", name="psv")
                for k in range(KT):
                    nc.tensor.matmul(
                        pv[:, 0:256],
                        xcols(k, s0, s0 + 128),
                        wv_sb[:, k, :],
                        start=(k == 0),
                        stop=(k == KT - 1),
                    )
                nc.vector.tensor_copy(v_sb[:, jt, :, 0:HD], pv[:, 0:256])

            def qk_units(c, jfs=(0, 1, 2, 3)):
                return [(lambda jf=jf: qkproj_unit(c, jf)) for jf in jfs]

            def v_units(c):
                return [(lambda sub=sub: vproj_unit(c, sub)) for sub in range(4)]

            # out-projection state shared between the interleaved units
            agx_t = {}
            ev_t = {}

            def ag_launch(c):
                """Store normalized oT chunk, trigger the AllGather, and
                queue the gathered load (chunks 0..2)."""
                cs = slice(c * CH, (c + 1) * CH)
                for pr in range(2):
                    nc.sync.dma_start(
                        agin_d[c][pr * 128 : (pr + 1) * 128, :], oT[pr][:, cs]
                    )
                nc.gpsimd.collective_compute(
                    "AllGather",
                    ALU.bypass,
                    replica_groups=GROUPS,
                    ins=[agin_d[c].opt()],
                    outs=[agout_d[c].opt()],
                )
                agx = agpool.tile([128, KT, CH], BF16, tag="agx", name=f"agx{c}")
                nc.sync.dma_start(agx[:, :, :], agout_r[c][:, :, :])
                agx_t[c] = agx

            def outproj_e2(c, e2):
                """Out-projection rows [128*e2, 128*(e2+1)) of this core's
                256 owned output rows, over chunk c's 512 columns."""
                po = pmm.tile([128, CH], F32, tag="pmm", name=f"poe{e2}")
                for k in range(KT):
                    nc.tensor.matmul(
                        po[:, :],
                        wo_sb[:, k, e2 * 128 : (e2 + 1) * 128],
                        agx_t[c][:, k, :],
                        start=(k == 0),
                        stop=(k == KT - 1),
                    )
                if e2 == 0:
                    ev_t[c] = epool.tile([128, 2, CH], BF16, tag="ev", name=f"ev{c}")
                nc.vector.tensor_scalar_add(
                    ev_t[c][:, e2, :], po[:, :], bias_sb[:, e2, :]
                )

            def outproj_store(c):
                nc.sync.dma_start(
                    out_r[:, :, c * CH : (c + 1) * CH], ev_t[c][:, :, :]
                )

            def outproj_units(c):
                return [
                    lambda: outproj_e2(c, 0),
                    lambda: outproj_e2(c, 1),
                    lambda: outproj_store(c),
                ]

            def attention(c, units0=(), units1=(), need=None, post_pr=None):
                """Attention for chunk c; `units0`/`units1` are independent
                work closures interleaved between pr=0 / pr=1 t-iterations so
                the PE stays dense through this ACT-heavy stretch.  All of a
                pr's units are flushed before its softmax-normalize ops, so
                the normalize (and the AllGather trigger it feeds) is never
                stuck behind filler work in the Vector FIFO.  DVE-heavy rope
                units belong in units0; light v/out-proj units in units1.
                `need(t)` gives the minimum number of units0 that must be
                emitted before the t-th iteration (kv units consumed by this
                same chunk)."""
                per_pr = [list(units0), list(units1)]
                cs = slice(c * CH, (c + 1) * CH)
                for pr in range(2):
                    units = per_pr[pr]
                    ntile = 4 * c + 4
                    # finish the units ~1.5 t-iterations before the pr ends
                    # so no filler DVE work precedes the normalize chain
                    slots = max(1, 2 * ntile - 3)
                    done = 0
                    emitted = 0
                    poA = pop.tile([HD + 1, CH], F32, tag="po", name="poA")
                    poB = pop.tile([HD + 1, CH], F32, tag="po", name="poB")
                    for t in range(ntile):
                        while emitted < (need(t) if need and pr == 0 else 0):
                            units[emitted]()
                            emitted += 1
                        ts_ = slice(t * 128, (t + 1) * 128)
                        # columns < off are causally dead for this j-tile
                        off = max(0, (t - 4 * c) * 128)
                        vs = slice(c * CH + off, (c + 1) * CH)
                        # both heads' scores in one 2-bank psum tile
                        s2 = psp.tile([128, 2 * CH], F32, tag="s", name="s2")
                        nc.tensor.matmul(
                            s2[:, off:CH], kT[pr][0:64, ts_], qT[pr][0:64, vs],
                            start=True, stop=True,
                        )
                        nc.tensor.matmul(
                            s2[:, CH + off : 2 * CH], kT[pr][64:128, ts_],
                            qT[pr][64:128, vs],
                            start=True, stop=True,
                        )
                        # one Exp over both heads' valid columns (3D AP)
                        at = apool.tile([128, 2, CH], BF16, tag="at")
                        nc.scalar.activation(
                            at[:, :, off:CH],
                            s2.rearrange("p (h n) -> p h n", h=2)[:, :, off:CH],
                            AF.Exp,
                            scale=SCALE,
                        )
                        if t >= 4 * c:  # diagonal-crossing tile
                            nc.vector.tensor_mul(
                                at[:, 0, off : off + 128],
                                at[:, 0, off : off + 128],
                                tri_sb[:, :],
                            )
                            nc.vector.tensor_mul(
                                at[:, 1, off : off + 128],
                                at[:, 1, off : off + 128],
                                tri_sb[:, :],
                            )
                        for hl, po_ in enumerate((poA, poB)):
                            nc.tensor.matmul(
                                po_[:, off:CH],
                                v_sb[:, t, 2 * pr + hl, :],
                                at[:, hl, off:CH],
                                start=(t == 0),
                                stop=(t == ntile - 1),
                            )
                            done += 1
                            want = min(len(units), (done * len(units)) // slots)
                            while emitted < want:
                                units[emitted]()
                                emitted += 1
                    for u in units[emitted:]:
                        u()
                    emitted = len(units)
                    # softmax denominators -> flat row -> one fast reciprocal
                    srow = spool.tile([1, 2 * CH], F32, tag="srow", name=f"srow{pr}")
                    nc.vector.tensor_copy(srow[0:1, 0:CH], poA[HD : HD + 1, :])
                    nc.vector.tensor_copy(srow[0:1, CH : 2 * CH], poB[HD : HD + 1, :])
                    rrow = spool.tile([1, 2 * CH], F32, tag="rrow", name=f"rrow{pr}")
                    nc.vector.reciprocal_approx_fast(rrow[:, :], srow[:, :])
                    nbs = []
                    for hl in range(2):
                        nb_ = apool.tile([64, CH], F32, tag=f"nb{hl}", name=f"nb{hl}")
                        nc.gpsimd.partition_broadcast(
                            nb_[:, :], rrow[0:1, hl * CH : (hl + 1) * CH]
                        )
                        nbs.append(nb_)
                    nc.vector.tensor_mul(oT[pr][0:64, cs], poA[0:64, :], nbs[0][:, :])
                    obuf = apool.tile([64, CH], BF16, tag="obuf")
                    nc.vector.tensor_mul(obuf[:, :], poB[0:64, :], nbs[1][:, :])
                    nc.vector.tensor_copy(oT[pr][64:128, cs], obuf[:, :])
                    if post_pr is not None and pr in post_pr:
                        post_pr[pr]()

            def ag3_launch(h):
                """Chunk-3 AllGather for head pair h (pr=h): rank r's 128
                rows land at agout3 rows [128r, 128r+128) = global feature
                block 256r + 128h, i.e. contraction k-tile 2r+h."""
                c3s = slice(3 * CH, S)
                nc.sync.dma_start(agin3_d[h][:, :], oT[h][:, c3s])
                nc.gpsimd.collective_compute(
                    "AllGather",
                    ALU.bypass,
                    replica_groups=GROUPS,
                    ins=[agin3_d[h].opt()],
                    outs=[agout3_d[h].opt()],
                )
                nc.sync.dma_start(agx3[h][:, :, :], agout3_r[h][:, :, :])

            def tail():
                """Chunk-3 out-projection: the even half of the contraction
                (head pairs 0-1 of every rank, gathered mid-attention) runs
                while AG3b is still in flight; the odd half + evacuation +
                store are all that remains after it lands."""
                po = psp.tile([128, 2 * CH], F32, tag="s", name="po3")
                ev = epool.tile([128, 2, CH], BF16, tag="ev", name="ev3")
                for e2 in (0, 1):
                    for ka in range(4):
                        nc.tensor.matmul(
                            po[:, e2 * CH : (e2 + 1) * CH],
                            wo_sb[:, 2 * ka, e2 * 128 : (e2 + 1) * 128],
                            agx3[0][:, ka, :],
                            start=(ka == 0),
                            stop=False,
                        )
                for e2 in (0, 1):
                    for kb in range(4):
                        nc.tensor.matmul(
                            po[:, e2 * CH : (e2 + 1) * CH],
                            wo_sb[:, 2 * kb + 1, e2 * 128 : (e2 + 1) * 128],
                            agx3[1][:, kb, :],
                            start=False,
                            stop=(kb == 3),
                        )
                    nc.vector.tensor_scalar_add(
                        ev[:, e2, :], po[:, e2 * CH : (e2 + 1) * CH], bias_sb[:, e2, :]
                    )
                nc.sync.dma_start(out_r[:, :, 3 * CH : S], ev[:, :, :])

            # schedule: attention(c) is ACT-heavy, so the PE work of other
            # phases is interleaved into it at t-iteration granularity.
            # DVE-heavy rope units fill pr=0; light v/out-proj units fill
            # pr=1, so each pr's normalize -> AllGather chain stays prompt.
            # The last (largest) attention chunk takes chunk 3's k/v
            # projections and chunk 2's out-projection as filler; chunk 3's
            # first head pair AllGathers mid-attention via the post_pr hook.
            for u in qk_units(0) + v_units(0):
                u()
            attention(0, units0=qk_units(1), units1=v_units(1))
            ag_launch(0)
            attention(1, units0=qk_units(2), units1=v_units(2) + outproj_units(0))
            ag_launch(1)
            attention(2, units0=qk_units(3, jfs=(0, 1)), units1=outproj_units(1))
            ag_launch(2)
            # kv units are consumed by attention(3) itself: unit i of
            # [kT0, kT1, v0..v3] must be emitted before the t-iteration
            # that reads it (t >= 12 reads v_{t-12} and both kT tiles)
            attention(
                3,
                units0=qk_units(3, jfs=(2, 3)) + v_units(3),
                units1=outproj_units(2),
                need=lambda t: 0 if t < 12 else 3 + (t - 12),
                post_pr={0: lambda: ag3_launch(0)},
            )
            ag3_launch(1)
            tail()

    return nc


_NC = None


def _get_nc():
    global _NC
    if _NC is None:
        nc = build_nc()
        nc.compile()
        _NC = nc
    return _NC


_TABLES = None


def _tables():
    global _TABLES
    if _TABLES is None:
        theta = 1.0 / ROPE_BASE ** (np.arange(0, HD, 2, dtype=np.float32) / HD)
        freqs = np.outer(np.arange(S, dtype=np.float32), theta)  # [S, 32]
        cos = np.cos(freqs).astype(np.float32)
        sin = np.sin(freqs).astype(np.float32)
        cosT = np.concatenate([cos, cos], axis=1).T  # [64, S]
        sinT = np.concatenate([-sin, sin], axis=1).T  # sign-absorbed
        cos2 = np.ascontiguousarray(np.concatenate([cosT, cosT], axis=0)).astype(BF)
        sin2 = np.ascontiguousarray(np.concatenate([sinT, sinT], axis=0)).astype(BF)
        trim = np.triu(np.ones((128, 128), dtype=np.float32)).astype(BF)
        _TABLES = (cos2, sin2, trim)
    return _TABLES


def make_in_maps(x, w_qkv, w_out, b_out):
    x = np.asarray(x, dtype=np.float32)
    w_qkv = np.asarray(w_qkv, dtype=np.float32)
    w_out = np.asarray(w_out, dtype=np.float32)
    b_out = np.asarray(b_out, dtype=np.float32)
    cos2, sin2, trim = _tables()
    xTs = [np.ascontiguousarray(x[b].T.astype(BF)) for b in range(B)]
    in_maps = []
    for core in range(NCORES):
        b, hg = core // 4, core % 4
        heads = np.arange(HPC * hg, HPC * hg + HPC)
        qrows = np.concatenate([np.arange(h * HD, (h + 1) * HD) for h in heads])
        krows = qrows + H * HD
        vrows = qrows + 2 * H * HD
        wqkT = w_qkv[np.concatenate([qrows, krows])].T.astype(BF)  # [1024, 512]
        # [jf, p, k, m]: stationary tile for feature block jf, contraction k
        wqk4 = np.ascontiguousarray(
            wqkT.reshape(KT, 128, 4, 128).transpose(2, 1, 0, 3)
        ).reshape(4 * 128, KT, 128)
        wvT = np.ascontiguousarray(w_qkv[vrows].T.astype(BF))  # [1024, 256]
        erows = np.arange(EOUT * hg, EOUT * hg + EOUT)
        woT = np.ascontiguousarray(w_out[erows].T.astype(BF))  # [1024, 256]
        bias2 = np.ascontiguousarray(
            b_out[erows].reshape(EOUT, 1).astype(np.float32)
        )
        in_maps.append(
            {
                "xT": xTs[b],
                "wqk4": wqk4,
                "wvT": wvT,
                "woT": woT,
                "cos2": cos2,
                "sin2": sin2,
                "trim": trim,
                "bias2": bias2,
            }
        )
    return in_maps


def assemble_out(results):
    out = np.empty((B, S, D), dtype=np.float32)
    for b in range(B):
        outT = np.concatenate(
            [np.asarray(results[4 * b + r]["out"]).astype(np.float32) for r in range(4)],
            axis=0,
        )  # [1024, 2048] = out[b].T
        out[b] = outT.T
    return out


def kernel(x, w_qkv, w_out, b_out):
    nc = _get_nc()
    in_maps = make_in_maps(x, w_qkv, w_out, b_out)
    res = run_bass_kernel_spmd(nc, in_maps, core_ids=list(range(NCORES)))
    return assemble_out(res.results)


if __name__ == "__main__":
    rng = np.random.default_rng(0)
    x = rng.standard_normal((B, S, D), dtype=np.float32)
    w_qkv = rng.standard_normal((3 * D, D), dtype=np.float32) * 0.02
    w_out = rng.standard_normal((D, D), dtype=np.float32) / 32.0
    b_out = np.zeros(D, dtype=np.float32)
    out = kernel(x, w_qkv, w_out, b_out)
    print("out", out.shape, out.dtype, float(np.abs(out).mean()))


# revision 20
# speedup vs baseline: 1.0071x; 1.0071x over previous
"""Trainium2 Bass kernel: multi-head causal attention with RoPE.

Problem: x[2, 2048, 1024], w_qkv[3072, 1024], w_out[1024, 1024], b_out[1024];
16 heads, head_dim 64, causal softmax attention with rotate-half RoPE.

Sharding: 8 cores = 2 batch groups x 4 head-groups.  Core c handles batch
b = c // 4 and heads 4*(c%4) .. 4*(c%4)+3.  Each core computes q/k/v
projections for its 4 heads, RoPE, causal flash-style attention with
softmax, then the cores in a batch group AllGather the (normalized)
attention outputs per seq chunk ([256, 512] bf16 slices -> [1024, 512]),
and every core computes the FULL out-projection for its own 256 output
rows locally.  No ReduceScatter of 4x-larger partials, no output copies:
the AllGather moves 4x fewer bytes than the partial-sum RS would, and the
final tail only waits on one small AG of the last chunk's second head
pair.

On-chip layout notes:
 - Everything keeps seq-len on the free axis and features on partitions
   (q/k/v are computed directly in transposed [feat, s] layout), so attention
   scores come out as scoresT [j, i] and no transposes are ever needed.
 - Softmax skips the max-subtraction (scores are O(1) by construction),
   so exp() happens directly on the QK^T PSUM tile, and the denominators are
   accumulated by an all-ones row appended to v in the AV matmul.
 - bf16 operands everywhere on the PE; fp32 PSUM accumulation.
 - 2 heads per 128-partition span -> automatic PE row packing for the K=64
   QK^T matmuls (tile_position derived from base partitions).
 - ScalarE runs ONLY Exp (no activation-table switching); all evacuation
   copies run on VectorE.
 - The chunk loop is software-pipelined: projections for chunk c+1 and the
   out-projection of chunk c-1 are interleaved into attention(c) at
   t-iteration granularity so the PE never stalls on the softmax-normalize
   chain.
 - Startup: per-jf qkv weight slices go on the Scalar HWDGE queue while
   per-k x chunk-0 slices go on the Sync queue, so the first matmul starts
   as soon as the first two small DMAs land.
"""

import os
import sys

import numpy as np

for _p in ("/opt/trn_rl_repo", "/opt/pypackages"):
    if os.path.isdir(_p) and _p not in sys.path:
        sys.path.append(_p)

import ml_dtypes  # noqa: E402

import concourse.bass as bass  # noqa: E402,F401
import concourse.mybir as mybir  # noqa: E402
import concourse.tile as tile  # noqa: E402
from concourse import bacc  # noqa: E402
from concourse.bass_utils import run_bass_kernel_spmd  # noqa: E402

BF = ml_dtypes.bfloat16
F32 = mybir.dt.float32
BF16 = mybir.dt.bfloat16
AF = mybir.ActivationFunctionType
ALU = mybir.AluOpType

B, S, D, H, HD = 2, 2048, 1024, 16, 64
NCORES = 8
HPC = 4  # heads per core
GROUPS = [[0, 1, 2, 3], [4, 5, 6, 7]]
CH = 512  # seq chunk width
NCH = S // CH  # 4
KT = D // 128  # 8 contraction tiles for the projections
NJT = S // 128  # 16 key tiles
EOUT = D // 4  # 256 output-feature rows owned by each core
ROPE_BASE = 10000.0
SCALE = 1.0 / 8.0  # 1/sqrt(64)


def build_nc():
    nc = bacc.Bacc("TRN2", target_bir_lowering=False, debug=False, num_devices=NCORES)

    xT_d = nc.dram_tensor("xT", [D, S], BF16, kind="ExternalInput").ap()
    wqk4_d = nc.dram_tensor("wqk4", [4 * 128, KT, 128], BF16, kind="ExternalInput").ap()
    wv_d = nc.dram_tensor("wvT", [D, HPC * HD], BF16, kind="ExternalInput").ap()
    wo_d = nc.dram_tensor("woT", [D, EOUT], BF16, kind="ExternalInput").ap()
    cos_d = nc.dram_tensor("cos2", [128, S], BF16, kind="ExternalInput").ap()
    sin_d = nc.dram_tensor("sin2", [128, S], BF16, kind="ExternalInput").ap()
    tri_d = nc.dram_tensor("trim", [128, 128], BF16, kind="ExternalInput").ap()
    bias_d = nc.dram_tensor("bias2", [EOUT, 1], F32, kind="ExternalInput").ap()
    out_d = nc.dram_tensor("out", [EOUT, S], BF16, kind="ExternalOutput").ap()

    # AllGather buffers: chunks 0-2 gather all 4 heads at once; chunk 3 is
    # split by head pair so the first AG fires while pr=1 is still computing
    agin_d = [
        nc.dram_tensor(f"agin{c}", [2 * 128, CH], BF16).ap() for c in range(NCH - 1)
    ]
    agout_d = [
        nc.dram_tensor(f"agout{c}", [D, CH], BF16).ap() for c in range(NCH - 1)
    ]
    agin3_d = [nc.dram_tensor(f"agin3{h}", [128, CH], BF16).ap() for h in range(2)]
    agout3_d = [
        nc.dram_tensor(f"agout3{h}", [4 * 128, CH], BF16).ap() for h in range(2)
    ]
    # tiny dummy collective fired at startup: absorbs the ~20us
    # first-collective warmup (ncfw/credit init) while the CC stream is idle
    agdum_in = nc.dram_tensor("agdumi", [128, 4], BF16).ap()
    agdum_out = nc.dram_tensor("agdumo", [4 * 128, 4], BF16).ap()

    xT_r = xT_d.rearrange("(k p) s -> p k s", p=128)
    wv_r = wv_d.rearrange("(k p) j -> p k j", p=128)
    wo_r = wo_d.rearrange("(k p) e -> p k e", p=128)
    out_r = out_d.rearrange("(t p) s -> p t s", p=128)
    agout_r = [a.rearrange("(k p) s -> p k s", p=128) for a in agout_d]
    agout3_r = [a.rearrange("(k p) s -> p k s", p=128) for a in agout3_d]

    with tile.TileContext(nc) as tc:
        with (
            tc.tile_pool(name="const", bufs=1) as cpool,
            tc.tile_pool(name="rope", bufs=3) as rpool,
            tc.tile_pool(name="attn", bufs=3) as apool,
            tc.tile_pool(name="evac", bufs=3) as epool,
            tc.tile_pool(name="sums", bufs=4) as spool,
            tc.tile_pool(name="agx", bufs=2) as agpool,
            tc.tile_pool(name="pmm", bufs=1, space="PSUM") as pmm,
            tc.tile_pool(name="ps", bufs=2, space="PSUM") as psp,
            tc.tile_pool(name="po", bufs=3, space="PSUM") as pop,
        ):
            # ---- startup loads.  Scalar HWDGE queue: qkv weights (per-jf
            # slices so the first matmul only waits on one 256KB transfer).
            # Sync HWDGE queue: x chunk 0 per-k slices, then the x rest as
            # one large transfer.  GpSimd SWDGE queue: rope tables, mask,
            # out-proj weights, bias. ----
            wqk_sb = cpool.tile([128, 4, KT, 128], BF16, tag="wqk")
            for jf in range(4):
                nc.scalar.dma_start(
                    wqk_sb[:, jf, :, :], wqk4_d[jf * 128 : (jf + 1) * 128, :, :]
                )
            wv_sb = cpool.tile([128, KT, 256], BF16, tag="wv")
            nc.scalar.dma_start(wv_sb[:, :, :], wv_r[:, :, :])

            xc0 = cpool.tile([128, KT, CH], BF16, tag="xc0")
            for k in range(KT):
                nc.sync.dma_start(xc0[:, k, :], xT_r[:, k, 0:CH])
            xrest = cpool.tile([128, KT, S - CH], BF16, tag="xrest")
            nc.sync.dma_start(xrest[:, :, :], xT_r[:, :, CH:S])

            def xcols(k, lo, hi):
                """x tile AP for contraction tile k, seq columns [lo, hi)."""
                if hi <= CH:
                    return xc0[:, k, lo:hi]
                return xrest[:, k, lo - CH : hi - CH]

            nc.gpsimd.collective_compute(
                "AllGather",
                ALU.bypass,
                replica_groups=GROUPS,
                ins=[agdum_in.opt()],
                outs=[agdum_out.opt()],
            )
            cos_sb = cpool.tile([128, S], BF16, tag="cos")
            nc.gpsimd.dma_start(cos_sb[:, :], cos_d)
            sin_sb = cpool.tile([128, S], BF16, tag="sin")
            nc.gpsimd.dma_start(sin_sb[:, :], sin_d)
            tri_sb = cpool.tile([128, 128], BF16, tag="tri")
            nc.gpsimd.dma_start(tri_sb[:, :], tri_d)
            wo_sb = cpool.tile([128, KT, EOUT], BF16, tag="wo")
            nc.gpsimd.dma_start(wo_sb[:, :, :], wo_r[:, :, :])
            bias_sb = cpool.tile([128, 2, 1], F32, tag="bias")
            nc.gpsimd.dma_start(bias_sb[:, :, :], bias_d.rearrange("(t p) o -> p t o", p=128))

            # ---- persistent activations (bf16) ----
            qT = [cpool.tile([128, S], BF16, tag=f"qT{i}", name=f"qT{i}") for i in range(2)]
            kT = [cpool.tile([128, S], BF16, tag=f"kT{i}", name=f"kT{i}") for i in range(2)]
            oT = [cpool.tile([128, S], BF16, tag=f"oT{i}", name=f"oT{i}") for i in range(2)]
            # v with an appended ones-column per head: [j_part, jt, head, 65]
            v_sb = cpool.tile([128, NJT, HPC, HD + 1], BF16, tag="v")
            nc.vector.memset(v_sb[:, :, :, HD : HD + 1], 1.0)
            agx3 = [
                cpool.tile([128, 4, CH], BF16, tag=f"agx3{h}", name=f"agx3{h}")
                for h in range(2)
            ]

            def qkproj_unit(c, jf):
                cs = slice(c * CH, (c + 1) * CH)
                dest = (qT[0], qT[1], kT[0], kT[1])[jf]
                ps = pmm.tile([128, CH], F32, tag="pmm", name="psqk")
                for k in range(KT):
                    nc.tensor.matmul(
                        ps[:, :],
                        wqk_sb[:, jf, k, :],
                        xcols(k, c * CH, (c + 1) * CH),
                        start=(k == 0),
                        stop=(k == KT - 1),
                    )
                qraw = rpool.tile([128, CH], BF16, tag="qraw")
                nc.vector.tensor_copy(qraw[:, :], ps[:, :])
                # rotate-half: qsw[r] = qraw[r ^ 32] (sign absorbed in sin2)
                qsw = rpool.tile([128, CH], BF16, tag="qsw")
                for h2 in (0, 64):
                    nc.vector.tensor_copy(
                        qsw[h2 : h2 + 32, :], qraw[h2 + 32 : h2 + 64, :]
                    )
                    nc.vector.tensor_copy(
                        qsw[h2 + 32 : h2 + 64, :], qraw[h2 : h2 + 32, :]
                    )
                t1 = rpool.tile([128, CH], BF16, tag="t1")
                nc.vector.tensor_mul(t1[:, :], qraw[:, :], cos_sb[:, cs])
                t2 = rpool.tile([128, CH], BF16, tag="t2")
                nc.vector.tensor_mul(t2[:, :], qsw[:, :], sin_sb[:, cs])
                nc.vector.tensor_add(dest[:, cs], t1[:, :], t2[:, :])

            def vproj_unit(c, sub):
                jt = 4 * c + sub
                s0 = c * CH + sub * 128
                pv = pmm.tile([128, CH], F32, tag="pmm", name="psv")
                for k in range(KT):
                    nc.tensor.matmul(
                        pv[:, 0:256],
                        xcols(k, s0, s0 + 128),
                        wv_sb[:, k, :],
                        start=(k == 0),
                        stop=(k == KT - 1),
                    )
                nc.vector.tensor_copy(v_sb[:, jt, :, 0:HD], pv[:, 0:256])

            def qk_units(c, jfs=(0, 1, 2, 3)):
                return [(lambda jf=jf: qkproj_unit(c, jf)) for jf in jfs]

            def v_units(c):
                return [(lambda sub=sub: vproj_unit(c, sub)) for sub in range(4)]

            # out-projection state shared between the interleaved units
            agx_t = {}
            ev_t = {}

            def ag_launch(c):
                """Store normalized oT chunk, trigger the AllGather, and
                queue the gathered load (chunks 0..2)."""
                cs = slice(c * CH, (c + 1) * CH)
                for pr in range(2):
                    # scalar HWDGE queue: never queues behind an agx load
                    nc.scalar.dma_start(
                        agin_d[c][pr * 128 : (pr + 1) * 128, :], oT[pr][:, cs]
                    )
                nc.gpsimd.collective_compute(
                    "AllGather",
                    ALU.bypass,
                    replica_groups=GROUPS,
                    ins=[agin_d[c].opt()],
                    outs=[agout_d[c].opt()],
                )
                agx = agpool.tile([128, KT, CH], BF16, tag="agx", name=f"agx{c}")
                nc.sync.dma_start(agx[:, :, :], agout_r[c][:, :, :])
                agx_t[c] = agx

            def outproj_e2(c, e2):
                """Out-projection rows [128*e2, 128*(e2+1)) of this core's
                256 owned output rows, over chunk c's 512 columns."""
                po = pmm.tile([128, CH], F32, tag="pmm", name=f"poe{e2}")
                for k in range(KT):
                    nc.tensor.matmul(
                        po[:, :],
                        wo_sb[:, k, e2 * 128 : (e2 + 1) * 128],
                        agx_t[c][:, k, :],
                        start=(k == 0),
                        stop=(k == KT - 1),
                    )
                if e2 == 0:
                    ev_t[c] = epool.tile([128, 2, CH], BF16, tag="ev", name=f"ev{c}")
                nc.vector.tensor_scalar_add(
                    ev_t[c][:, e2, :], po[:, :], bias_sb[:, e2, :]
                )

            def outproj_store(c):
                nc.sync.dma_start(
                    out_r[:, :, c * CH : (c + 1) * CH], ev_t[c][:, :, :]
                )

            def outproj_units(c):
                return [
                    lambda: outproj_e2(c, 0),
                    lambda: outproj_e2(c, 1),
                    lambda: outproj_store(c),
                ]

            def attention(c, units0=(), units1=(), need=None, post_pr=None):
                """Attention for chunk c; `units0`/`units1` are independent
                work closures interleaved between pr=0 / pr=1 t-iterations so
                the PE stays dense through this ACT-heavy stretch.  All of a
                pr's units are flushed before its softmax-normalize ops, so
                the normalize (and the AllGather trigger it feeds) is never
                stuck behind filler work in the Vector FIFO.  DVE-heavy rope
                units belong in units0; light v/out-proj units in units1.
                `need(t)` gives the minimum number of units0 that must be
                emitted before the t-th iteration (kv units consumed by this
                same chunk)."""
                per_pr = [list(units0), list(units1)]
                cs = slice(c * CH, (c + 1) * CH)
                for pr in range(2):
                    units = per_pr[pr]
                    ntile = 4 * c + 4
                    # finish the units ~1.5 t-iterations before the pr ends
                    # so no filler DVE work precedes the normalize chain
                    slots = max(1, 2 * ntile - 3)
                    done = 0
                    emitted = 0
                    poA = pop.tile([HD + 1, CH], F32, tag="po", name="poA")
                    poB = pop.tile([HD + 1, CH], F32, tag="po", name="poB")
                    for t in range(ntile):
                        while emitted < (need(t) if need and pr == 0 else 0):
                            units[emitted]()
                            emitted += 1
                        ts_ = slice(t * 128, (t + 1) * 128)
                        # columns < off are causally dead for this j-tile
                        off = max(0, (t - 4 * c) * 128)
                        vs = slice(c * CH + off, (c + 1) * CH)
                        # both heads' scores in one 2-bank psum tile
                        s2 = psp.tile([128, 2 * CH], F32, tag="s", name="s2")
                        nc.tensor.matmul(
                            s2[:, off:CH], kT[pr][0:64, ts_], qT[pr][0:64, vs],
                            start=True, stop=True,
                        )
                        nc.tensor.matmul(
                            s2[:, CH + off : 2 * CH], kT[pr][64:128, ts_],
                            qT[pr][64:128, vs],
                            start=True, stop=True,
                        )
                        # one Exp over both heads' valid columns (3D AP)
                        at = apool.tile([128, 2, CH], BF16, tag="at")
                        nc.scalar.activation(
                            at[:, :, off:CH],
                            s2.rearrange("p (h n) -> p h n", h=2)[:, :, off:CH],
                            AF.Exp,
                            scale=SCALE,
                        )
                        if t >= 4 * c:  # diagonal-crossing tile
                            nc.vector.tensor_mul(
                                at[:, 0, off : off + 128],
                                at[:, 0, off : off + 128],
                                tri_sb[:, :],
                            )
                            nc.vector.tensor_mul(
                                at[:, 1, off : off + 128],
                                at[:, 1, off : off + 128],
                                tri_sb[:, :],
                            )
                        for hl, po_ in enumerate((poA, poB)):
                            nc.tensor.matmul(
                                po_[:, off:CH],
                                v_sb[:, t, 2 * pr + hl, :],
                                at[:, hl, off:CH],
                                start=(t == 0),
                                stop=(t == ntile - 1),
                            )
                            done += 1
                            want = min(len(units), (done * len(units)) // slots)
                            while emitted < want:
                                units[emitted]()
                                emitted += 1
                    for u in units[emitted:]:
                        u()
                    emitted = len(units)
                    # softmax denominators -> flat row -> one fast reciprocal
                    srow = spool.tile([1, 2 * CH], F32, tag="srow", name=f"srow{pr}")
                    nc.vector.tensor_copy(srow[0:1, 0:CH], poA[HD : HD + 1, :])
                    nc.vector.tensor_copy(srow[0:1, CH : 2 * CH], poB[HD : HD + 1, :])
                    rrow = spool.tile([1, 2 * CH], F32, tag="rrow", name=f"rrow{pr}")
                    nc.vector.reciprocal_approx_fast(rrow[:, :], srow[:, :])
                    nbs = []
                    for hl in range(2):
                        nb_ = apool.tile([64, CH], F32, tag=f"nb{hl}", name=f"nb{hl}")
                        nc.gpsimd.partition_broadcast(
                            nb_[:, :], rrow[0:1, hl * CH : (hl + 1) * CH]
                        )
                        nbs.append(nb_)
                    nc.vector.tensor_mul(oT[pr][0:64, cs], poA[0:64, :], nbs[0][:, :])
                    obuf = apool.tile([64, CH], BF16, tag="obuf")
                    nc.vector.tensor_mul(obuf[:, :], poB[0:64, :], nbs[1][:, :])
                    nc.vector.tensor_copy(oT[pr][64:128, cs], obuf[:, :])
                    if post_pr is not None and pr in post_pr:
                        post_pr[pr]()

            def ag3_launch(h):
                """Chunk-3 AllGather for head pair h (pr=h): rank r's 128
                rows land at agout3 rows [128r, 128r+128) = global feature
                block 256r + 128h, i.e. contraction k-tile 2r+h."""
                c3s = slice(3 * CH, S)
                nc.scalar.dma_start(agin3_d[h][:, :], oT[h][:, c3s])
                nc.gpsimd.collective_compute(
                    "AllGather",
                    ALU.bypass,
                    replica_groups=GROUPS,
                    ins=[agin3_d[h].opt()],
                    outs=[agout3_d[h].opt()],
                )
                nc.sync.dma_start(agx3[h][:, :, :], agout3_r[h][:, :, :])

            def tail():
                """Chunk-3 out-projection: chunk 2's out-projection and the
                even contraction half (head pairs 0-1 of every rank, gathered
                mid-attention) keep the PE busy while AG3b is still in
                flight; the odd half + evacuation + store are all that
                remains after it lands."""
                po = psp.tile([128, 2 * CH], F32, tag="s", name="po3")
                ev = epool.tile([128, 2, CH], BF16, tag="ev", name="ev3")
                op2 = outproj_units(2)
                for e2 in (0, 1):
                    op2[e2]()
                    for ka in range(4):
                        nc.tensor.matmul(
                            po[:, e2 * CH : (e2 + 1) * CH],
                            wo_sb[:, 2 * ka, e2 * 128 : (e2 + 1) * 128],
                            agx3[0][:, ka, :],
                            start=(ka == 0),
                            stop=False,
                        )
                op2[2]()  # chunk-2 out store
                for e2 in (0, 1):
                    for kb in range(4):
                        nc.tensor.matmul(
                            po[:, e2 * CH : (e2 + 1) * CH],
                            wo_sb[:, 2 * kb + 1, e2 * 128 : (e2 + 1) * 128],
                            agx3[1][:, kb, :],
                            start=False,
                            stop=(kb == 3),
                        )
                    nc.vector.tensor_scalar_add(
                        ev[:, e2, :], po[:, e2 * CH : (e2 + 1) * CH], bias_sb[:, e2, :]
                    )
                nc.sync.dma_start(out_r[:, :, 3 * CH : S], ev[:, :, :])

            # schedule: attention(c) is ACT-heavy, so the PE work of other
            # phases is interleaved into it at t-iteration granularity.
            # DVE-heavy rope units fill pr=0; light v/out-proj units fill
            # pr=1, so each pr's normalize -> AllGather chain stays prompt.
            # The last (largest) attention chunk takes chunk 3's k/v
            # projections and chunk 2's out-projection as filler; chunk 3's
            # first head pair AllGathers mid-attention via the post_pr hook.
            # out-proj(c) fillers go into attention(c+2) so their matmuls
            # never enter the in-order PE queue before AG(c) has landed
            for u in qk_units(0) + v_units(0):
                u()
            attention(0, units0=qk_units(1), units1=v_units(1))
            ag_launch(0)
            attention(1, units0=qk_units(2), units1=v_units(2))
            ag_launch(1)
            attention(2, units0=qk_units(3), units1=v_units(3) + outproj_units(0))
            ag_launch(2)
            attention(
                3,
                units1=outproj_units(1),
                post_pr={0: lambda: ag3_launch(0)},
            )
            ag3_launch(1)
            tail()

    return nc


_NC = None


def _get_nc():
    global _NC
    if _NC is None:
        nc = build_nc()
        nc.compile()
        _NC = nc
    return _NC


_TABLES = None


def _tables():
    global _TABLES
    if _TABLES is None:
        theta = 1.0 / ROPE_BASE ** (np.arange(0, HD, 2, dtype=np.float32) / HD)
        freqs = np.outer(np.arange(S, dtype=np.float32), theta)  # [S, 32]
        cos = np.cos(freqs).astype(np.float32)
        sin = np.sin(freqs).astype(np.float32)
        cosT = np.concatenate([cos, cos], axis=1).T  # [64, S]
        sinT = np.concatenate([-sin, sin], axis=1).T  # sign-absorbed
        cos2 = np.ascontiguousarray(np.concatenate([cosT, cosT], axis=0)).astype(BF)
        sin2 = np.ascontiguousarray(np.concatenate([sinT, sinT], axis=0)).astype(BF)
        trim = np.triu(np.ones((128, 128), dtype=np.float32)).astype(BF)
        _TABLES = (cos2, sin2, trim)
    return _TABLES


def make_in_maps(x, w_qkv, w_out, b_out):
    x = np.asarray(x, dtype=np.float32)
    w_qkv = np.asarray(w_qkv, dtype=np.float32)
    w_out = np.asarray(w_out, dtype=np.float32)
    b_out = np.asarray(b_out, dtype=np.float32)
    cos2, sin2, trim = _tables()
    xTs = [np.ascontiguousarray(x[b].T.astype(BF)) for b in range(B)]
    in_maps = []
    for core in range(NCORES):
        b, hg = core // 4, core % 4
        heads = np.arange(HPC * hg, HPC * hg + HPC)
        qrows = np.concatenate([np.arange(h * HD, (h + 1) * HD) for h in heads])
        krows = qrows + H * HD
        vrows = qrows + 2 * H * HD
        wqkT = w_qkv[np.concatenate([qrows, krows])].T.astype(BF)  # [1024, 512]
        # [jf, p, k, m]: stationary tile for feature block jf, contraction k
        wqk4 = np.ascontiguousarray(
            wqkT.reshape(KT, 128, 4, 128).transpose(2, 1, 0, 3)
        ).reshape(4 * 128, KT, 128)
        wvT = np.ascontiguousarray(w_qkv[vrows].T.astype(BF))  # [1024, 256]
        erows = np.arange(EOUT * hg, EOUT * hg + EOUT)
        woT = np.ascontiguousarray(w_out[erows].T.astype(BF))  # [1024, 256]
        bias2 = np.ascontiguousarray(
            b_out[erows].reshape(EOUT, 1).astype(np.float32)
        )
        in_maps.append(
            {
                "xT": xTs[b],
                "wqk4": wqk4,
                "wvT": wvT,
                "woT": woT,
                "cos2": cos2,
                "sin2": sin2,
                "trim": trim,
                "bias2": bias2,
            }
        )
    return in_maps


def assemble_out(results):
    out = np.empty((B, S, D), dtype=np.float32)
    for b in range(B):
        outT = np.concatenate(
            [np.asarray(results[4 * b + r]["out"]).astype(np.float32) for r in range(4)],
            axis=0,
        )  # [1024, 2048] = out[b].T
        out[b] = outT.T
    return out


def kernel(x, w_qkv, w_out, b_out):
    nc = _get_nc()
    in_maps = make_in_maps(x, w_qkv, w_out, b_out)
    res = run_bass_kernel_spmd(nc, in_maps, core_ids=list(range(NCORES)))
    return assemble_out(res.results)


if __name__ == "__main__":
    rng = np.random.default_rng(0)
    x = rng.standard_normal((B, S, D), dtype=np.float32)
    w_qkv = rng.standard_normal((3 * D, D), dtype=np.float32) * 0.02
    w_out = rng.standard_normal((D, D), dtype=np.float32) / 32.0
    b_out = np.zeros(D, dtype=np.float32)
    out = kernel(x, w_qkv, w_out, b_out)
    print("out", out.shape, out.dtype, float(np.abs(out).mean()))
